# revision 12
# baseline (speedup 1.0000x reference)
"""Soft-DTW layer (band-limited, gamma=1) as a Bass/Tile kernel on 8 TRN2 cores.

Problem: x [64, 512] f32, protos [32, 64] f32 -> out [64, 32, 1] f32
  out[b, f, 0] = softDTW(C[b,f]) / T, C[b,f][i,j] = (x[b,i]-protos[f,j])^2,
  Sakoe-Chiba band |i/511 - j/63| <= 0.2, out-of-band = BIG.

Exp-space DP (per (b,f) problem): E'(i,j) = e^{a*i - D(i,j)} satisfies, with
G = e^{a-C},
    E'(i,j) = G(i,j) * (E'(i-1,j) + E'(i-1,j-1) + e^{-a} * E'(i,j-1))
Column sweep j=0..63 over the in-band run [a_j, b_j) of the 216-row window
[8j-104, 8j+112).

v3 structure (three-engine pipeline, exact band with NO zeroing passes):
  PE:  w_j = ECA*E_{j-1}[8+v] + E_{j-1}[7+v] via two accumulating
       diagonal-matmuls (bf16 identity weights; the shift lives in the rhs
       access pattern) into PSUM.  Per-problem renorm scales ride the
       diagonals (scaled weight copies, rebuilt per renorm event).
  ACT: G production: per-column Square (bias -p_j) from PSUM-replicated x,
       per-chunk Exp; also the renorm magnitude sums (Copy+accum) and the
       scaled-weight rebuilds.
  DVE: per-column per-group scans E_j = scan(w_j(PSUM), G) -> bf16, over
       exactly the in-band run.  The two groups' chains interleave, so PE's
       w-matmuls hide behind the sibling group's scan.
Band edges come for free: scan initial=0 resets the left border, and the
monotone drift of (a_j, b_j) means all out-of-run taps land on never-written
zero-initialized cells (verified structurally).

Sharding: data-parallel over batch. Core c handles b in [8c, 8c+8); its 256
(b,f) problems sit as 2 groups of 128 partitions:
  partition p, group g -> b = 8c + 4g + p//32, f = p%32.
"""

import numpy as np

import concourse.bass as bass
import concourse.bacc as bacc
import concourse.mybir as mybir
import concourse.tile as tile
from concourse.bass_utils import run_bass_kernel_spmd

T, K, L = 512, 64, 216
NCORES = 8
BOFF = 104        # column j covers rows [8j-104, 8j+112)
GS = 448          # per-column G stride: [g0 216 | g1 216 | pad 16]
EW = 224          # per-group E-buffer width (216 + 8 tap slack)
A = 0.75          # rescale slope per row
FB = 8            # renorm every FB columns
ECA = float(np.exp(-A))
F32 = mybir.dt.float32
BF16 = mybir.dt.bfloat16
AL = mybir.AluOpType
AF = mybir.ActivationFunctionType

# chunk sizes for the G pipeline (sum = K); ramp keeps ACT ahead of DVE
CHUNKS = [1, 2, 4, 6, 8, 8, 8, 8, 8, 11]


def _band_runs():
    """Per column: (a, b) with in-band window cells exactly [a, b);
    bit-identical to the reference band mask (float32 arithmetic)."""
    ii = np.arange(T, dtype=np.float32) / np.float32(T - 1)
    jj = np.arange(K, dtype=np.float32) / np.float32(K - 1)
    mask = np.abs(ii[:, None] - jj[None, :]) <= np.float32(0.2)
    runs = []
    for j in range(K):
        rows = 8 * j - BOFF + np.arange(L)
        inb = np.zeros(L, bool)
        valid = (rows >= 0) & (rows < T)
        inb[valid] = mask[rows[valid], j]
        idx = np.where(inb)[0]
        runs.append((int(idx[0]), int(idx[-1]) + 1))
    return runs


BAND = _band_runs()
NFB = sum(1 for j in range(K) if j % FB == 0 and j > 0)


def _ap(t, offset, dims):
    """Custom free-dim access pattern on tile t: dims = [[step, count], ...]
    (element units), keeping the partition dim."""
    ap = t[:, 0:1].copy()
    ap.ap = ap.ap[:1] + [[int(s), int(n)] for s, n in dims]
    ap.offset = int(offset)
    return ap


def build_nc():
    nc = bacc.Bacc("TRN2")
    xs = nc.dram_tensor("xs", [8, T], F32, kind="ExternalInput")
    pr = nc.dram_tensor("protos", [32, K], F32, kind="ExternalInput")
    out = nc.dram_tensor("out", [128, 2], F32, kind="ExternalOutput")

    with tile.TileContext(nc) as tc:
        with tc.tile_pool(name="main", bufs=1) as pool, \
                tc.psum_pool(name="xps", bufs=1) as ppool:
            xs8 = pool.tile([8, T], F32)             # raw x rows
            Wt = pool.tile([8, 256], F32)            # x-replication weights
            P = ppool.tile([128, 2 * T], F32)        # replicated x (2 banks)
            prt = pool.tile([128, K], F32)           # protos row per problem
            negp = pool.tile([128, K], F32)          # -protos (Square bias)
            # banded G, one tile per chunk (exact RAW/WAR tracking)
            Gt = [pool.tile([128, W * GS], F32, name=f"G{k}")
                  for k, W in enumerate(CHUNKS)]
            # E buffers: both groups in one tile (g at offset 224*g), bf16
            colA = pool.tile([128, 2 * EW], BF16)
            colB = pool.tile([128, 2 * EW], BF16)
            # identity weights (base only; renorm scales cprev in place)
            Ib = pool.tile([128, 128], BF16)
            Ie = pool.tile([128, 128], BF16)
            # w in PSUM: (group, ping) -> tile
            wps = [ppool.tile([128, 208], F32, name=f"w{i}") for i in range(4)]
            sl = pool.tile([128, 2], F32)            # renorm sums (g)
            mxb = pool.tile([128, 2 * NFB], F32)     # applied scales s (k, g)
            lnmx = pool.tile([128, 2 * NFB], F32)    # ln of scales
            rsc = pool.tile([128, L], F32)           # renorm-sum copy target
            ef = pool.tile([128, 2], F32)
            efe = pool.tile([128, 2], mybir.dt.int32)
            eff = pool.tile([128, 2], F32)
            efm = pool.tile([128, 2], mybir.dt.int32)
            lnmant = pool.tile([128, 2], F32)
            lnef = pool.tile([128, 2], F32)
            lnS = pool.tile([128, 2], F32)
            tt = pool.tile([128, 2], F32)
            osb = pool.tile([128, 2], F32)
            acon = pool.tile([128, 1], F32)          # bias const A for Exp
            scr = pool.tile([128, 8], F32)           # scratch

            # ---- init ----
            nc.sync.dma_start(xs8[:, :], xs[:, :])
            psrc = pr[:, :].unsqueeze(0).broadcast_to([4, 32, K])
            nc.sync.dma_start(prt[:, :], psrc)
            # x-replication weights: W[p, 128g + m] = 1 iff p == 4g + m//32
            nc.vector.memset(Wt[:, :], 1.0)
            nc.gpsimd.affine_select(
                Wt[:, :], Wt[:, :], pattern=[[1, 256]], base=0,
                compare_op=AL.is_ge, fill=0.0, channel_multiplier=-32)
            nc.gpsimd.affine_select(
                Wt[:, :], Wt[:, :], pattern=[[-1, 256]], base=31,
                compare_op=AL.is_ge, fill=0.0, channel_multiplier=32)
            # identity (diagonal) weights
            nc.vector.memset(Ib[:, :], 1.0)
            nc.gpsimd.affine_select(
                Ib[:, :], Ib[:, :], pattern=[[1, 128]], base=0,
                compare_op=AL.is_ge, fill=0.0, channel_multiplier=-1)
            nc.gpsimd.affine_select(
                Ib[:, :], Ib[:, :], pattern=[[-1, 128]], base=0,
                compare_op=AL.is_ge, fill=0.0, channel_multiplier=1)
            nc.vector.tensor_scalar(Ie[:, :], Ib[:, :], ECA, None, op0=AL.mult)
            nc.vector.memset(colA[:, :], 0.0)
            nc.vector.memset(colB[:, :], 0.0)
            # virtual-corner seed E'(-1,-1) = e^{-a} at window position 111
            nc.vector.memset(colA[:, 111:112], ECA)
            nc.vector.memset(colA[:, EW + 111:EW + 112], ECA)
            nc.vector.memset(acon[:, :], A)
            nc.vector.tensor_scalar(negp[:, :], prt[:, :], -1.0, None,
                                    op0=AL.mult)
            nc.vector.tensor_copy(scr[0:8, 0:1], xs8[:, 0:1])  # touch x DMA
            tc.no_sync_barrier()
            # replicate x into PSUM: P[p, g*T + t] = xs[4g + p//32, t].
            # Rows [0,128) first (all chunks 0-1 need); the rest overlaps
            # the early columns' DP (emitted at chunk 1 below).
            nc.tensor.matmul(P[:, 0:128], Wt[:, 0:128], xs8[:, 0:128])
            nc.tensor.matmul(P[:, T:T + 128], Wt[:, 128:256], xs8[:, 0:128])
            tc.no_sync_barrier()

            # ---- pipelined G production + column DP ----
            fb_k = 0
            cprev, ccur = colA, colB
            j0 = 0
            for ck, W in enumerate(CHUNKS):
                cols = range(j0, j0 + W)
                G = Gt[ck]
                if ck == 1:
                    # rest of the x replication, off the startup chain
                    nc.tensor.matmul(P[:, 128:T], Wt[:, 0:128],
                                     xs8[:, 128:T])
                    nc.tensor.matmul(P[:, T + 128:2 * T], Wt[:, 128:256],
                                     xs8[:, 128:T])
                # ACT: C = (x - p_j)^2, in-band cells of both groups
                for j in cols:
                    a, b = BAND[j]
                    gout = _ap(G, (j - j0) * GS + a, [[L, 2], [1, b - a]])
                    xin = _ap(P, 8 * j - BOFF + a, [[T, 2], [1, b - a]])
                    nc.scalar.activation(gout, xin, AF.Square,
                                         bias=negp[:, j:j + 1], scale=1.0)
                # ACT: G = exp(A - C) for the whole chunk (out-of-run cells
                # hold garbage-exp values; nothing ever reads them)
                gch = _ap(G, 0, [[GS, W], [1, 2 * L]])
                nc.scalar.activation(gch, gch, AF.Exp,
                                     bias=acon[:, :], scale=-1.0)

                for j in cols:
                    a, b = BAND[j]
                    n = b - a
                    renorm = (j % FB == 0 and j > 0)
                    if renorm:
                        # renorm: scale from column j-2's magnitude (summed
                        # off the critical path on ACT; ccur holds E_{j-2}).
                        # The scale is applied to the PSUM w AFTER the
                        # matmuls (w*s == taps of s*E), so PE never stalls
                        # behind the scale application.
                        ap2, bp2 = BAND[j - 2]
                        for g in (0, 1):
                            nc.scalar.activation(
                                rsc[:, 0:bp2 - ap2],
                                ccur[:, EW * g + ap2:EW * g + bp2],
                                AF.Copy, accum_out=sl[:, g:g + 1])
                        nc.vector.reciprocal(
                            mxb[:, 2 * fb_k:2 * fb_k + 2], sl[:, :])
                    for g in (0, 1):
                        wp = wps[2 * g + (j % 2)]
                        o = EW * g
                        nc.tensor.matmul(wp[:, 0:n], Ie[:, :],
                                         cprev[:, o + 8 + a:o + 8 + b],
                                         start=True, stop=False,
                                         skip_group_check=True)
                    for g in (0, 1):
                        wp = wps[2 * g + (j % 2)]
                        o = EW * g
                        nc.tensor.matmul(wp[:, 0:n], Ib[:, :],
                                         cprev[:, o + 7 + a:o + 7 + b],
                                         start=False, stop=True,
                                         skip_group_check=True)
                    for g in (0, 1):
                        wp = wps[2 * g + (j % 2)]
                        o = EW * g
                        if renorm:
                            sap = mxb[:, 2 * fb_k + g:2 * fb_k + g + 1]
                            nc.vector.tensor_scalar(
                                wp[:, 0:n], wp[:, 0:n], sap, None,
                                op0=AL.mult)
                        nc.vector.tensor_tensor_scan(
                            ccur[:, o + a:o + b], wp[:, 0:n],
                            G[:, (j - j0) * GS + L * g + a:
                               (j - j0) * GS + L * g + b],
                            0.0, op0=AL.add, op1=AL.mult)
                    if renorm:
                        fb_k += 1
                    cprev, ccur = ccur, cprev
                j0 += W

            # deferred renorm logs; dummy Ln anchored on the last G tile so
            # ACT loads the Ln table after the exps
            nc.scalar.activation(scr[:, 2:3], Gt[-1][:, 0:1], AF.Ln)
            nc.scalar.activation(lnmx[:, :], mxb[:, :], AF.Ln)

            last = cprev  # column 63 buffer
            # ---- extraction: D = a*511 + sum(ln s) - ln(E'fin); out = D/T --
            nc.vector.tensor_reduce(
                lnS[:, :], lnmx[:, :].rearrange("p (k g) -> p g k", g=2),
                axis=mybir.AxisListType.X, op=AL.add)
            nc.vector.tensor_copy(ef[:, 0:1], last[:, 111:112])
            nc.vector.tensor_copy(ef[:, 1:2], last[:, EW + 111:EW + 112])
            # frexp-style log: ln(ef) = Ln(mantissa) + (exp - 127)*ln2
            eiv = ef[:, :].bitcast(mybir.dt.int32)
            nc.vector.tensor_scalar(efe[:, :], eiv, 23, None,
                                    op0=AL.arith_shift_right)
            nc.vector.tensor_copy(eff[:, :], efe[:, :])   # int -> float value
            nc.vector.tensor_scalar(efm[:, :], eiv, 0x007FFFFF, 0x3F800000,
                                    op0=AL.bitwise_and,
                                    op1=AL.bitwise_or)
            nc.scalar.activation(lnmant[:, :], efm[:, :].bitcast(F32),
                                 AF.Ln)
            nc.vector.scalar_tensor_tensor(
                lnef[:, :], eff[:, :], float(np.log(2.0)), lnmant[:, :],
                op0=AL.mult, op1=AL.add)
            # tt = ln(E'fin) - lnS ; out = -tt/T + (A(T-1) + 127 ln2)/T
            nc.vector.tensor_tensor(tt[:, :], lnef[:, :], lnS[:, :],
                                    op=AL.subtract)
            nc.vector.tensor_scalar(
                osb[:, :], tt[:, :], float(-1.0 / T),
                float((A * (T - 1) + 127.0 * np.log(2.0)) / T),
                op0=AL.mult, op1=AL.add)
            nc.sync.dma_start(out[:, :], osb[:, :])

    nc.compile()
    return nc


_NC = None


def _get_nc():
    global _NC
    if _NC is None:
        _NC = build_nc()
    return _NC


def kernel(x: np.ndarray, protos: np.ndarray) -> np.ndarray:
    x = np.ascontiguousarray(x, dtype=np.float32)
    protos = np.ascontiguousarray(protos, dtype=np.float32)
    nc = _get_nc()
    in_maps = [
        {"xs": x[8 * c: 8 * c + 8], "protos": protos} for c in range(NCORES)
    ]
    res = run_bass_kernel_spmd(nc, in_maps, core_ids=list(range(NCORES)))
    out = np.empty((64, 32, 1), dtype=np.float32)
    for c in range(NCORES):
        r = res.results[c]["out"]                 # [128, 2]
        blk = r.reshape(4, 32, 2).transpose(2, 0, 1)  # [g, bb, f]
        out[8 * c: 8 * c + 8, :, 0] = blk.reshape(8, 32)
    return out


# revision 14
# speedup vs baseline: 1.0159x; 1.0159x over previous
"""Soft-DTW layer (band-limited, gamma=1) as a Bass/Tile kernel on 8 TRN2 cores.

Problem: x [64, 512] f32, protos [32, 64] f32 -> out [64, 32, 1] f32
  out[b, f, 0] = softDTW(C[b,f]) / T, C[b,f][i,j] = (x[b,i]-protos[f,j])^2,
  Sakoe-Chiba band |i/511 - j/63| <= 0.2, out-of-band = BIG.

Exp-space DP (per (b,f) problem): E'(i,j) = e^{a*i - D(i,j)} satisfies, with
G = e^{a-C},
    E'(i,j) = G(i,j) * (E'(i-1,j) + E'(i-1,j-1) + e^{-a} * E'(i,j-1))
Column sweep j=0..63 over the in-band run [a_j, b_j) of the 216-row window
[8j-104, 8j+112).

v3 structure (three-engine pipeline, exact band with NO zeroing passes):
  PE:  w_j = ECA*E_{j-1}[8+v] + E_{j-1}[7+v] via two accumulating
       diagonal-matmuls (bf16 identity weights; the shift lives in the rhs
       access pattern) into PSUM.  Per-problem renorm scales ride the
       diagonals (scaled weight copies, rebuilt per renorm event).
  ACT: G production: per-column Square (bias -p_j) from PSUM-replicated x,
       per-chunk Exp; also the renorm magnitude sums (Copy+accum) and the
       scaled-weight rebuilds.
  DVE: per-column per-group scans E_j = scan(w_j(PSUM), G) -> bf16, over
       exactly the in-band run.  The two groups' chains interleave, so PE's
       w-matmuls hide behind the sibling group's scan.
Band edges come for free: scan initial=0 resets the left border, and the
monotone drift of (a_j, b_j) means all out-of-run taps land on never-written
zero-initialized cells (verified structurally).

Sharding: data-parallel over batch. Core c handles b in [8c, 8c+8); its 256
(b,f) problems sit as 2 groups of 128 partitions:
  partition p, group g -> b = 8c + 4g + p//32, f = p%32.
"""

import numpy as np

import concourse.bass as bass
import concourse.bacc as bacc
import concourse.mybir as mybir
import concourse.tile as tile
from concourse.bass_utils import run_bass_kernel_spmd

T, K, L = 512, 64, 216
NCORES = 8
BOFF = 104        # column j covers rows [8j-104, 8j+112)
GS = 448          # per-column G stride: [g0 216 | g1 216 | pad 16]
EW = 224          # per-group E-buffer width (216 + 8 tap slack)
A = 0.75          # rescale slope per row
FB = 8            # renorm every FB columns
ECA = float(np.exp(-A))
F32 = mybir.dt.float32
BF16 = mybir.dt.bfloat16
AL = mybir.AluOpType
AF = mybir.ActivationFunctionType

# chunk sizes for the G pipeline (sum = K); ramp keeps ACT ahead of DVE
CHUNKS = [1, 2, 4, 6, 8, 8, 8, 8, 8, 11]


def _band_runs():
    """Per column: (a, b) with in-band window cells exactly [a, b);
    bit-identical to the reference band mask (float32 arithmetic)."""
    ii = np.arange(T, dtype=np.float32) / np.float32(T - 1)
    jj = np.arange(K, dtype=np.float32) / np.float32(K - 1)
    mask = np.abs(ii[:, None] - jj[None, :]) <= np.float32(0.2)
    runs = []
    for j in range(K):
        rows = 8 * j - BOFF + np.arange(L)
        inb = np.zeros(L, bool)
        valid = (rows >= 0) & (rows < T)
        inb[valid] = mask[rows[valid], j]
        idx = np.where(inb)[0]
        runs.append((int(idx[0]), int(idx[-1]) + 1))
    return runs


BAND = _band_runs()
NFB = sum(1 for j in range(K) if j % FB == 0 and j > 0)


def _ap(t, offset, dims):
    """Custom free-dim access pattern on tile t: dims = [[step, count], ...]
    (element units), keeping the partition dim."""
    ap = t[:, 0:1].copy()
    ap.ap = ap.ap[:1] + [[int(s), int(n)] for s, n in dims]
    ap.offset = int(offset)
    return ap


def build_nc():
    nc = bacc.Bacc("TRN2")
    xs = nc.dram_tensor("xs", [8, T], F32, kind="ExternalInput")
    pr = nc.dram_tensor("protos", [32, K], F32, kind="ExternalInput")
    out = nc.dram_tensor("out", [128, 2], F32, kind="ExternalOutput")

    with tile.TileContext(nc) as tc:
        with tc.tile_pool(name="main", bufs=1) as pool, \
                tc.psum_pool(name="xps", bufs=1) as ppool:
            xs8 = pool.tile([8, T], F32)             # raw x rows
            Wt = pool.tile([8, 256], F32)            # x-replication weights
            P = ppool.tile([128, 2 * T], F32)        # replicated x (2 banks)
            prt = pool.tile([128, K], F32)           # protos row per problem
            negp = pool.tile([128, K], F32)          # -protos (Square bias)
            # banded G, one tile per chunk (exact RAW/WAR tracking)
            Gt = [pool.tile([128, W * GS], F32, name=f"G{k}")
                  for k, W in enumerate(CHUNKS)]
            # E buffers: both groups in one tile (g at offset 224*g), bf16
            colA = pool.tile([128, 2 * EW], BF16)
            colB = pool.tile([128, 2 * EW], BF16)
            # identity weights (base only; renorm scales cprev in place)
            Ib = pool.tile([128, 128], BF16)
            Ie = pool.tile([128, 128], BF16)
            # w in PSUM: (group, ping) -> tile
            wps = [ppool.tile([128, 208], F32, name=f"w{i}") for i in range(4)]
            sl = pool.tile([128, 2], F32)            # renorm sums (g)
            mxb = pool.tile([128, 2 * NFB], F32)     # applied scales s (k, g)
            lnmx = pool.tile([128, 2 * NFB], F32)    # ln of scales
            rsc = pool.tile([128, L], F32)           # renorm-sum copy target
            ef = pool.tile([128, 2], F32)
            efe = pool.tile([128, 2], mybir.dt.int32)
            eff = pool.tile([128, 2], F32)
            efm = pool.tile([128, 2], mybir.dt.int32)
            lnmant = pool.tile([128, 2], F32)
            lnef = pool.tile([128, 2], F32)
            lnS = pool.tile([128, 2], F32)
            tt = pool.tile([128, 2], F32)
            osb = pool.tile([128, 2], F32)
            acon = pool.tile([128, 1], F32)          # bias const A for Exp
            scr = pool.tile([128, 8], F32)           # scratch

            # ---- init ----
            nc.sync.dma_start(xs8[:, :], xs[:, :])
            psrc = pr[:, :].unsqueeze(0).broadcast_to([4, 32, K])
            nc.sync.dma_start(prt[:, :], psrc)
            # x-replication weights: W[p, 128g + m] = 1 iff p == 4g + m//32
            nc.vector.memset(Wt[:, :], 1.0)
            nc.gpsimd.affine_select(
                Wt[:, :], Wt[:, :], pattern=[[1, 256]], base=0,
                compare_op=AL.is_ge, fill=0.0, channel_multiplier=-32)
            nc.gpsimd.affine_select(
                Wt[:, :], Wt[:, :], pattern=[[-1, 256]], base=31,
                compare_op=AL.is_ge, fill=0.0, channel_multiplier=32)
            # identity (diagonal) weights
            nc.vector.memset(Ib[:, :], 1.0)
            nc.gpsimd.affine_select(
                Ib[:, :], Ib[:, :], pattern=[[1, 128]], base=0,
                compare_op=AL.is_ge, fill=0.0, channel_multiplier=-1)
            nc.gpsimd.affine_select(
                Ib[:, :], Ib[:, :], pattern=[[-1, 128]], base=0,
                compare_op=AL.is_ge, fill=0.0, channel_multiplier=1)
            nc.vector.tensor_scalar(Ie[:, :], Ib[:, :], ECA, None, op0=AL.mult)
            nc.vector.memset(colA[:, :], 0.0)
            nc.vector.memset(colB[:, :], 0.0)
            # virtual-corner seed E'(-1,-1) = e^{-a} at window position 111
            nc.vector.memset(colA[:, 111:112], ECA)
            nc.vector.memset(colA[:, EW + 111:EW + 112], ECA)
            nc.vector.memset(acon[:, :], A)
            nc.vector.tensor_scalar(negp[:, :], prt[:, :], -1.0, None,
                                    op0=AL.mult)
            nc.vector.tensor_copy(scr[0:8, 0:1], xs8[:, 0:1])  # touch x DMA
            tc.no_sync_barrier()
            # replicate x into PSUM: P[p, g*T + t] = xs[4g + p//32, t]
            nc.tensor.matmul(P[:, 0:T], Wt[:, 0:128], xs8[:, :])
            nc.tensor.matmul(P[:, T:2 * T], Wt[:, 128:256], xs8[:, :])
            tc.no_sync_barrier()

            # ---- pipelined G production + column DP ----
            fb_k = 0
            cprev, ccur = colA, colB
            j0 = 0
            for ck, W in enumerate(CHUNKS):
                cols = range(j0, j0 + W)
                G = Gt[ck]
                # ACT: C = (x - p_j)^2, in-band cells of both groups
                for j in cols:
                    a, b = BAND[j]
                    gout = _ap(G, (j - j0) * GS + a, [[L, 2], [1, b - a]])
                    xin = _ap(P, 8 * j - BOFF + a, [[T, 2], [1, b - a]])
                    nc.scalar.activation(gout, xin, AF.Square,
                                         bias=negp[:, j:j + 1], scale=1.0)
                # ACT: G = exp(A - C) for the whole chunk (out-of-run cells
                # hold garbage-exp values; nothing ever reads them)
                gch = _ap(G, 0, [[GS, W], [1, 2 * L]])
                nc.scalar.activation(gch, gch, AF.Exp,
                                     bias=acon[:, :], scale=-1.0)

                for j in cols:
                    a, b = BAND[j]
                    n = b - a
                    renorm = (j % FB == 0 and j > 0)
                    if renorm:
                        # renorm: scale from column j-2's magnitude (summed
                        # off the critical path on ACT; ccur holds E_{j-2}).
                        # The scale is applied to the PSUM w AFTER the
                        # matmuls (w*s == taps of s*E), so PE never stalls
                        # behind the scale application.
                        ap2, bp2 = BAND[j - 2]
                        for g in (0, 1):
                            nc.scalar.activation(
                                rsc[:, 0:bp2 - ap2],
                                ccur[:, EW * g + ap2:EW * g + bp2],
                                AF.Copy, accum_out=sl[:, g:g + 1])
                        nc.vector.reciprocal(
                            mxb[:, 2 * fb_k:2 * fb_k + 2], sl[:, :])
                    for g in (0, 1):
                        wp = wps[2 * g + (j % 2)]
                        o = EW * g
                        nc.tensor.matmul(wp[:, 0:n], Ie[:, :],
                                         cprev[:, o + 8 + a:o + 8 + b],
                                         start=True, stop=False,
                                         skip_group_check=True)
                    for g in (0, 1):
                        wp = wps[2 * g + (j % 2)]
                        o = EW * g
                        nc.tensor.matmul(wp[:, 0:n], Ib[:, :],
                                         cprev[:, o + 7 + a:o + 7 + b],
                                         start=False, stop=True,
                                         skip_group_check=True)
                    for g in (0, 1):
                        wp = wps[2 * g + (j % 2)]
                        o = EW * g
                        if renorm:
                            sap = mxb[:, 2 * fb_k + g:2 * fb_k + g + 1]
                            nc.vector.tensor_scalar(
                                wp[:, 0:n], wp[:, 0:n], sap, None,
                                op0=AL.mult)
                        nc.vector.tensor_tensor_scan(
                            ccur[:, o + a:o + b], wp[:, 0:n],
                            G[:, (j - j0) * GS + L * g + a:
                               (j - j0) * GS + L * g + b],
                            0.0, op0=AL.add, op1=AL.mult)
                    if renorm:
                        fb_k += 1
                    cprev, ccur = ccur, cprev
                j0 += W

            # deferred renorm logs; dummy Ln anchored on the last G tile so
            # ACT loads the Ln table after the exps
            nc.scalar.activation(scr[:, 2:3], Gt[-1][:, 0:1], AF.Ln)
            nc.scalar.activation(lnmx[:, :], mxb[:, :], AF.Ln)

            last = cprev  # column 63 buffer
            # ---- extraction: D = a*511 + sum(ln s) - ln(E'fin); out = D/T --
            nc.vector.tensor_reduce(
                lnS[:, :], lnmx[:, :].rearrange("p (k g) -> p g k", g=2),
                axis=mybir.AxisListType.X, op=AL.add)
            nc.vector.tensor_copy(ef[:, 0:1], last[:, 111:112])
            nc.vector.tensor_copy(ef[:, 1:2], last[:, EW + 111:EW + 112])
            # frexp-style log: ln(ef) = Ln(mantissa) + (exp - 127)*ln2
            eiv = ef[:, :].bitcast(mybir.dt.int32)
            nc.vector.tensor_scalar(efe[:, :], eiv, 23, None,
                                    op0=AL.arith_shift_right)
            nc.vector.tensor_copy(eff[:, :], efe[:, :])   # int -> float value
            nc.vector.tensor_scalar(efm[:, :], eiv, 0x007FFFFF, 0x3F800000,
                                    op0=AL.bitwise_and,
                                    op1=AL.bitwise_or)
            nc.scalar.activation(lnmant[:, :], efm[:, :].bitcast(F32),
                                 AF.Ln)
            nc.vector.scalar_tensor_tensor(
                lnef[:, :], eff[:, :], float(np.log(2.0)), lnmant[:, :],
                op0=AL.mult, op1=AL.add)
            # tt = ln(E'fin) - lnS ; out = -tt/T + (A(T-1) + 127 ln2)/T
            nc.vector.tensor_tensor(tt[:, :], lnef[:, :], lnS[:, :],
                                    op=AL.subtract)
            nc.vector.tensor_scalar(
                osb[:, :], tt[:, :], float(-1.0 / T),
                float((A * (T - 1) + 127.0 * np.log(2.0)) / T),
                op0=AL.mult, op1=AL.add)
            nc.sync.dma_start(out[:, :], osb[:, :])

    nc.compile()
    return nc


_NC = None


def _get_nc():
    global _NC
    if _NC is None:
        _NC = build_nc()
    return _NC


def kernel(x: np.ndarray, protos: np.ndarray) -> np.ndarray:
    x = np.ascontiguousarray(x, dtype=np.float32)
    protos = np.ascontiguousarray(protos, dtype=np.float32)
    nc = _get_nc()
    in_maps = [
        {"xs": x[8 * c: 8 * c + 8], "protos": protos} for c in range(NCORES)
    ]
    res = run_bass_kernel_spmd(nc, in_maps, core_ids=list(range(NCORES)))
    out = np.empty((64, 32, 1), dtype=np.float32)
    for c in range(NCORES):
        r = res.results[c]["out"]                 # [128, 2]
        blk = r.reshape(4, 32, 2).transpose(2, 0, 1)  # [g, bb, f]
        out[8 * c: 8 * c + 8, :, 0] = blk.reshape(8, 32)
    return out
